# revision 16
# baseline (speedup 1.0000x reference)
"""5-layer DGL-style GraphConv (AwA2Conv) on 8 Trainium2 NeuronCores.

Math per layer (norm='both'):
    out = D_in^{-1/2} A D_out^{-1/2} (h) @ W + b     (+ leaky_relu except last)

The per-edge weight w_e = dinv_out[src]*dinv_in[dst] is folded into
block-sparse "S" matrices (128 edges x 128 dst one-hot-weighted) so the
sparse aggregation becomes PE matmuls over dma_gather'ed edge rows.
Aggregation runs at min(Fin, Fout) per layer (matmul commutes with the
linear aggregation). With lhsT = gathered rows, the aggregate comes out
TRANSPOSED [F, dst], which is exactly the lhsT layout the next dense matmul
needs — the whole network runs with zero explicit transposes.

Distribution: nodes sharded 6250/core; x and weights replicated via in_maps;
h exchanged between layers with AllGather; each core aggregates + transforms
its own 6250 destination nodes.
"""

import numpy as np

import concourse.bass as bass
import concourse.bacc as bacc
import concourse.mybir as mybir
import concourse.tile as tile
from concourse.bass_utils import run_bass_kernel_spmd

N_NODES = 50000
N_EDGES = 250000
NC = 8
NPC = N_NODES // NC  # 6250
P = 128
N_TILES = (NPC + P - 1) // P  # 49, last tile has 106 dsts
DIMS = [300, 1024, 512, 256, 128, 2048]
NEG_SLOPE = 0.01
LO_LIM = 32768  # int16 gather index limit

import ml_dtypes

F32 = mybir.dt.float32
BF16 = mybir.dt.bfloat16
DT = BF16                     # compute/storage dtype for activations/weights/S
NPDT = ml_dtypes.bfloat16
I16 = mybir.dt.int16
LRELU = mybir.ActivationFunctionType.Lrelu

LAYER_FA = [300, 512, 256, 128, 128]       # aggregation width
LAYER_FA_PAD = [384, 512, 256, 128, 128]   # table row width (256B multiple)
GROUP_T = 2                                # dst-tiles per dma_gather call


def _ceil_div(a, b):
    return (a + b - 1) // b


# ----------------------------------------------------------------------------
# Host-side graph preprocessing
# ----------------------------------------------------------------------------

def _prep(edge_index, x=None):
    """Partition edges by dst core/tile, split lo/hi src, pad to 16-granular
    per-tile schedules (max across cores -> one SPMD program).

    Returns (sched_lo, sched_hi, per_core): sched_* are per-tile padded idx
    counts (multiples of 16); per_core holds wrapped int16 gather indices,
    S matrices (128-row chunks, zero-padded), and the pre-gathered x rows
    for layer 1 (host-side halo materialization of the replicated input).
    """
    GRAN = 16
    src = np.asarray(edge_index[0], dtype=np.int64)
    dst = np.asarray(edge_index[1], dtype=np.int64)
    out_deg = np.bincount(src, minlength=N_NODES).astype(np.float32)
    in_deg = np.bincount(dst, minlength=N_NODES).astype(np.float32)
    dinv_out = 1.0 / np.sqrt(np.maximum(out_deg, 1.0))
    dinv_in = 1.0 / np.sqrt(np.maximum(in_deg, 1.0))
    w = (dinv_out[src] * dinv_in[dst]).astype(np.float32)

    lo = src < LO_LIM
    core_all = dst // NPC
    tile_all = (dst % NPC) // P
    key = ((core_all * N_TILES + tile_all) * 2 + (~lo).astype(np.int64))
    order = np.lexsort((src, key))
    src_s, w_s, dst_s, key_s = src[order], w[order], dst[order], key[order]
    bounds = np.searchsorted(key_s, np.arange(NC * N_TILES * 2 + 1))

    n_lo = np.zeros((NC, N_TILES), dtype=np.int64)
    n_hi = np.zeros((NC, N_TILES), dtype=np.int64)
    for c in range(NC):
        for t in range(N_TILES):
            k = (c * N_TILES + t) * 2
            n_lo[c, t] = bounds[k + 1] - bounds[k]
            n_hi[c, t] = bounds[k + 2] - bounds[k + 1]

    sched_lo = np.maximum(
        np.ceil(n_lo.max(axis=0) / GRAN).astype(np.int64), 1) * GRAN
    sched_hi = np.ceil(n_hi.max(axis=0) / GRAN).astype(np.int64) * GRAN

    xb = None if x is None else np.asarray(x, dtype=np.float32)

    per_core = []
    for c in range(NC):
        idx_parts = {True: [], False: []}
        s_parts = {True: [], False: []}
        xg_parts = {True: [], False: []}
        for t in range(N_TILES):
            k = (c * N_TILES + t) * 2
            segs = (
                (True, sched_lo[t], bounds[k], bounds[k + 1]),
                (False, sched_hi[t], bounds[k + 1], bounds[k + 2]),
            )
            for islo, ni, a, b_ in segs:
                ni = int(ni)
                if ni == 0:
                    continue
                n_slots = _ceil_div(ni, P) * P  # chunk-padded (S/xg rows)
                ne = b_ - a
                assert ne <= ni
                idx = np.zeros(ni, dtype=np.int64)
                idx[:ne] = src_s[a:b_] - (0 if islo else LO_LIM)
                dstloc = np.full(n_slots, P, dtype=np.int64)
                dstloc[:ne] = dst_s[a:b_] % NPC - t * P
                wv = np.zeros(n_slots, dtype=np.float32)
                wv[:ne] = w_s[a:b_]
                S = np.zeros((n_slots, P), dtype=np.float32)
                valid = dstloc < P
                S[np.nonzero(valid)[0], dstloc[valid]] = wv[valid]
                s_parts[islo].append(S.reshape(-1, P, P))
                idx_parts[islo].append(idx.reshape(-1, 16).T.astype(np.int16))
                if xb is not None:
                    xg = np.zeros((n_slots, 384), dtype=NPDT)
                    xg[:ne, :300] = xb[src_s[a:b_]].astype(NPDT)
                    xg_parts[islo].append(xg.reshape(-1, P, 384))
        idx_lo = np.tile(np.concatenate(idx_parts[True], axis=1), (8, 1))
        s_lo = np.concatenate(s_parts[True], axis=0)
        if idx_parts[False]:
            idx_hi = np.tile(np.concatenate(idx_parts[False], axis=1), (8, 1))
            s_hi = np.concatenate(s_parts[False], axis=0)
        else:
            idx_hi = np.zeros((128, 1), dtype=np.int16)
            s_hi = np.zeros((1, P, P), dtype=np.float32)
        pc = dict(
            idx_lo=np.ascontiguousarray(idx_lo),
            idx_hi=np.ascontiguousarray(idx_hi),
            s_lo=np.ascontiguousarray(s_lo),
            s_hi=np.ascontiguousarray(s_hi),
        )
        if xb is not None:
            pc["xg_lo"] = np.ascontiguousarray(np.concatenate(xg_parts[True], axis=0))
            pc["xg_hi"] = (
                np.ascontiguousarray(np.concatenate(xg_parts[False], axis=0))
                if xg_parts[False]
                else np.zeros((1, P, 384), dtype=NPDT)
            )
        per_core.append(pc)
    return sched_lo, sched_hi, per_core


# ----------------------------------------------------------------------------
# Bass program builder (depends only on sched_lo / sched_hi)
# ----------------------------------------------------------------------------

def _build(sched_lo, sched_hi, debug=False):
    nc = bacc.Bacc("TRN2")
    ch_lo = np.ceil(sched_lo / P).astype(np.int64)   # chunks per tile
    ch_hi = np.ceil(sched_hi / P).astype(np.int64)
    idx_lo_cols = int(sched_lo.sum()) // 16
    idx_hi_cols = max(int(sched_hi.sum()) // 16, 1)
    tot_clo = int(ch_lo.sum())
    tot_chi = max(int(ch_hi.sum()), 1)
    offi_lo = np.concatenate([[0], np.cumsum(sched_lo // 16)]).astype(int)  # idx cols
    offi_hi = np.concatenate([[0], np.cumsum(sched_hi // 16)]).astype(int)
    offc_lo = np.concatenate([[0], np.cumsum(ch_lo)]).astype(int)           # chunks
    offc_hi = np.concatenate([[0], np.cumsum(ch_hi)]).astype(int)

    xg_lo_d = nc.declare_dram_parameter("xg_lo", [tot_clo, P, 384], DT, isOutput=False)
    xg_hi_d = nc.declare_dram_parameter("xg_hi", [tot_chi, P, 384], DT, isOutput=False)
    Ws, bs = [], []
    for i in range(5):
        fi, fo = DIMS[i], DIMS[i + 1]
        Ws.append(nc.declare_dram_parameter(f"W{i+1}", [fi, fo], DT, isOutput=False))
        bs.append(nc.declare_dram_parameter(f"b{i+1}", [fo, 1], F32, isOutput=False))
    b4r_d = nc.declare_dram_parameter("b4r", [1, 128], DT, isOutput=False)
    b5r_d = nc.declare_dram_parameter("b5r", [128, 2048], F32, isOutput=False)
    idx_lo_d = nc.declare_dram_parameter("idx_lo", [128, idx_lo_cols], I16, isOutput=False)
    idx_hi_d = nc.declare_dram_parameter("idx_hi", [128, idx_hi_cols], I16, isOutput=False)
    s_lo_d = nc.declare_dram_parameter("s_lo", [tot_clo, P, P], DT, isOutput=False)
    s_hi_d = nc.declare_dram_parameter("s_hi", [tot_chi, P, P], DT, isOutput=False)
    out_d = nc.declare_dram_parameter("out", [NPC, 2048], F32, isOutput=True)

    with tile.TileContext(nc) as tc:
        with (
            tc.tile_pool(name="dram", bufs=1, space="DRAM") as dram,
            tc.tile_pool(name="cpool", bufs=1) as cpool,
            tc.tile_pool(name="sb", bufs=2) as sb,
            tc.tile_pool(name="pagg", bufs=1, space="PSUM") as pagg,
            tc.tile_pool(name="pmm", bufs=2, space="PSUM") as pmm,
        ):
            # ---- internal DRAM ----
            h1T_d = dram.tile([1024, NPC], DT)
            h2T_d = dram.tile([512, NPC], DT)
            h3T_d = dram.tile([256, NPC], DT)
            g2_d = dram.tile([NPC, 512], DT)
            g3_d = dram.tile([NPC, 256], DT)
            g4_d = dram.tile([NPC, 128], DT)
            h4_d = dram.tile([NPC, 128], DT)
            T2 = dram.tile([N_NODES, 512], DT, addr_space="Shared")
            T3 = dram.tile([N_NODES, 256], DT, addr_space="Shared")
            T4 = dram.tile([N_NODES, 128], DT, addr_space="Shared")
            T5 = dram.tile([N_NODES, 128], DT, addr_space="Shared")

            # ---- resident SBUF ----
            idx_lo_sb = cpool.tile([128, idx_lo_cols], I16, name="idxlo")
            nc.sync.dma_start(idx_lo_sb[:], idx_lo_d[:])
            idx_hi_sb = cpool.tile([128, idx_hi_cols], I16, name="idxhi")
            nc.sync.dma_start(idx_hi_sb[:], idx_hi_d[:])
            aggT1_sb = [
                cpool.tile([P, NPC], DT, name=f"aggT1_{k}") for k in range(3)
            ]
            aggT5_sb = cpool.tile([P, NPC], DT, name="aggT5")
            ones_sb = cpool.tile([1, 128], DT, name="ones")
            nc.any.memset(ones_sb[:], 1.0)
            b4r_sb = cpool.tile([1, 128], DT, name="b4rsb")
            nc.sync.dma_start(b4r_sb[:], b4r_d[:])
            b5r_sb = cpool.tile([128, 2048], F32, name="b5rsb")
            nc.sync.dma_start(b5r_sb[:], b5r_d[:])

            rg = [list(range(NC))]

            def load_w(i):
                fi, fo = DIMS[i], DIMS[i + 1]
                ks = []
                for k in range(_ceil_div(fi, P)):
                    kk = min(P, fi - k * P)
                    t_ = cpool.tile([P, fo], DT, name=f"w{i}_{k}", tag=f"wk{k}")
                    nc.sync.dma_start(t_[:kk, :], Ws[i][k * P : k * P + kk, :])
                    ks.append((t_, kk))
                return ks

            def load_bcol(i):
                fo = DIMS[i + 1]
                nchunk = _ceil_div(fo, P)
                t_ = cpool.tile([P, 16], F32, name=f"bc{i}", tag="bcol")
                for m in range(nchunk):
                    mm = min(P, fo - m * P)
                    nc.sync.dma_start(t_[:mm, m : m + 1], bs[i][m * P : m * P + mm, :])
                return t_

            # ================= aggregation =================
            def aggregate(layer, table_ap, out_cb, node_major=False):
                """Gather + aggregate all dst tiles.

                layer 0 reads host-shipped pre-gathered x rows (xg_*) with a
                plain DMA; other layers dma_gather from table_ap. S matrices
                are zero-padded to 128-row chunks; gathers are 16-granular
                with partial-K matmuls on the last chunk of each tile.
                """
                fa = LAYER_FA[layer]
                fap = LAYER_FA_PAD[layer]
                nfc = _ceil_div(fa, P)
                for g0 in range(0, N_TILES, 4):
                    tiles = list(range(g0, min(g0 + 4, N_TILES)))
                    t0, t1 = tiles[0], tiles[-1]
                    # S loads, one per group (contiguous chunk ranges)
                    slo_sb = sb.tile([128, int(offc_lo[t1 + 1] - offc_lo[t0]), P], DT,
                                     name=f"slo_{layer}_{g0}", tag="slo")
                    nc.sync.dma_start(
                        slo_sb[:],
                        s_lo_d[offc_lo[t0] : offc_lo[t1 + 1]].rearrange("c p n -> p c n"),
                    )
                    g_chi = int(offc_hi[t1 + 1] - offc_hi[t0])
                    shi_sb = None
                    if g_chi > 0:
                        shi_sb = sb.tile([128, g_chi, P], DT, name=f"shi_{layer}_{g0}", tag="shi")
                        nc.sync.dma_start(
                            shi_sb[:],
                            s_hi_d[offc_hi[t0] : offc_hi[t1 + 1]].rearrange("c p n -> p c n"),
                        )
                    if layer == 0:
                        # pre-gathered rows: one DMA per group per half
                        clo_g = int(offc_lo[t1 + 1] - offc_lo[t0])
                        hg_lo_g = sb.tile([128, clo_g, fap], DT, name=f"hglo_{layer}_{g0}", tag="hglo")
                        nc.sync.dma_start(
                            hg_lo_g[:],
                            xg_lo_d[offc_lo[t0] : offc_lo[t1 + 1]].rearrange("c p n -> p c n"),
                        )
                        hg_hi_g = None
                        if g_chi > 0:
                            hg_hi_g = sb.tile([128, g_chi, fap], DT, name=f"hghi_{layer}_{g0}", tag="hghi")
                            nc.sync.dma_start(
                                hg_hi_g[:],
                                xg_hi_d[offc_hi[t0] : offc_hi[t1 + 1]].rearrange("c p n -> p c n"),
                            )
                    for t in tiles:
                        tw = min(P, NPC - t * P)
                        # chunk list: (hg_tile, local_chunk, S_tile, s_chunk, K)
                        chunks = []
                        if layer == 0:
                            ni = int(sched_lo[t])
                            for ci in range(int(ch_lo[t])):
                                chunks.append((hg_lo_g, int(offc_lo[t] - offc_lo[t0]) + ci,
                                               slo_sb, int(offc_lo[t] - offc_lo[t0]) + ci, P))
                            ni = int(sched_hi[t])
                            for ci in range(int(ch_hi[t])):
                                chunks.append((hg_hi_g, int(offc_hi[t] - offc_hi[t0]) + ci,
                                               shi_sb, int(offc_hi[t] - offc_hi[t0]) + ci, P))
                        else:
                            ni_lo = int(sched_lo[t])
                            hg_lo = sb.tile([128, int(ch_lo[t]), fap], DT,
                                            name=f"hglo_{layer}_{t}", tag="hglo")
                            nc.gpsimd.dma_gather(
                                hg_lo[:], table_ap,
                                idx_lo_sb[:, offi_lo[t] : offi_lo[t + 1]],
                                ni_lo, ni_lo, fap,
                            )
                            for ci in range(int(ch_lo[t])):
                                chunks.append((hg_lo, ci, slo_sb,
                                               int(offc_lo[t] - offc_lo[t0]) + ci,
                                               min(P, ni_lo - ci * P)))
                            ni_hi = int(sched_hi[t])
                            if ni_hi > 0:
                                hg_hi = sb.tile([128, int(ch_hi[t]), fap], DT,
                                                name=f"hghi_{layer}_{t}", tag="hghi")
                                nc.gpsimd.dma_gather(
                                    hg_hi[:], table_ap[LO_LIM:, :],
                                    idx_hi_sb[:, offi_hi[t] : offi_hi[t + 1]],
                                    ni_hi, ni_hi, fap,
                                )
                                for ci in range(int(ch_hi[t])):
                                    chunks.append((hg_hi, ci, shi_sb,
                                                   int(offc_hi[t] - offc_hi[t0]) + ci,
                                                   min(P, ni_hi - ci * P)))
                        # one PSUM bank per accumulation group: first_mm's
                        # has_written clear is (partition-row x bank)-granular
                        pts = [
                            pagg.tile([P, P], F32, name=f"pt_{layer}_{t}_{fc}",
                                      tag=f"pagg{fc}", space="PSUM",
                                      bufs=(2 if fc < 2 else 1))
                            for fc in range(nfc)
                        ]
                        nch = len(chunks)
                        if node_major:
                            for ci, (hg, hc, ssb, sc, K) in enumerate(chunks):
                                nc.tensor.matmul(
                                    pts[0][:, :fa], ssb[:K, sc, :], hg[:K, hc, :fa],
                                    start=(ci == 0), stop=False,
                                )
                            nc.tensor.matmul(  # += bias row
                                pts[0][:, :fa], ones_sb[:1, :], b4r_sb[:1, :fa],
                                start=False, stop=True,
                            )
                        else:
                            for ci, (hg, hc, ssb, sc, K) in enumerate(chunks):
                                for fc in range(nfc):
                                    fw = min(P, fa - fc * P)
                                    nc.tensor.matmul(
                                        pts[fc][:fw, :],
                                        hg[:K, hc, fc * P : fc * P + fw],
                                        ssb[:K, sc, :],
                                        start=(ci == 0), stop=(ci == nch - 1),
                                    )
                        out_cb(t, tw, pts)

            # transposed-agg eviction via staging tiles flushed every 4 dst-tiles
            def make_staged_out(layer, nfc, dst_dram, bias_col, lrelu):
                state = {"stages": None, "c0": 0}

                def flush(c0, cw):
                    for fc in range(nfc):
                        nc.scalar.dma_start(
                            dst_dram[fc * P : (fc + 1) * P, c0 : c0 + cw],
                            state["stages"][fc][:, :cw],
                        )

                def cb(t, tw, pts):
                    if t % 4 == 0:
                        state["stages"] = [
                            sb.tile([P, 512], DT, name=f"st_{layer}_{t}_{fc}", tag=f"st{fc}")
                            for fc in range(nfc)
                        ]
                        state["c0"] = t * P
                    col = (t % 4) * P
                    for fc in range(nfc):
                        if lrelu:
                            nc.scalar.activation(
                                state["stages"][fc][:, col : col + tw],
                                pts[fc][:, :tw],
                                LRELU, bias=bias_col[:, fc : fc + 1], alpha=NEG_SLOPE,
                            )
                        else:
                            nc.vector.tensor_copy(
                                state["stages"][fc][:, col : col + tw],
                                pts[fc][:, :tw],
                            )
                    if t % 4 == 3 or t == N_TILES - 1:
                        flush(state["c0"], t * P + tw - state["c0"])

                return cb

            # dense (node-major out): g[n, :] = hT[:, n]^T @ W (+bias) (+lrelu)
            def dense_n(li, hT_src, fi, fo, w_tiles, g_dst, bias_row, lrelu):
                nk = fi // P
                nn = _ceil_div(fo, 512)
                src_is_sb = hT_src.space == bass.MemorySpace.SBUF
                assert not src_is_sb or nk == 1
                for d0 in range(0, NPC, 512):
                    dw = min(512, NPC - d0)
                    if src_is_sb:
                        hsb = None
                    else:
                        hsb = sb.tile([128, nk, 512], DT, name=f"hsb_{li}_{d0}", tag="hsb")
                        nc.sync.dma_start(
                            hsb[:, :, :dw],
                            hT_src[:, d0 : d0 + dw].rearrange("(k p) c -> p k c", p=P),
                        )
                    for m4 in range(_ceil_div(dw, P)):
                        mw = min(P, dw - m4 * P)
                        if fo > 512:
                            ev = sb.tile([P, 2048], F32, name=f"oev_{li}_{d0}_{m4}", tag="oev")
                        else:
                            ev = sb.tile([P, 512], DT, name=f"ev_{li}_{d0}_{m4}", tag="ev")
                        for n in range(nn):
                            nw = min(512, fo - n * 512)
                            pm = pmm.tile([P, 512], F32, name=f"pm_{li}_{d0}_{m4}_{n}",
                                          tag="pmm", space="PSUM")
                            for k in range(nk):
                                lhs = (
                                    hT_src[:, d0 + m4 * P : d0 + m4 * P + mw]
                                    if src_is_sb
                                    else hsb[:, k, m4 * P : m4 * P + mw]
                                )
                                nc.tensor.matmul(
                                    pm[:mw, :nw],
                                    lhs,
                                    w_tiles[k][0][:w_tiles[k][1], n * 512 : n * 512 + nw],
                                    start=(k == 0), stop=(k == nk - 1),
                                )
                            if lrelu:
                                nc.scalar.activation(
                                    ev[:mw, n * 512 : n * 512 + nw], pm[:mw, :nw],
                                    LRELU, alpha=NEG_SLOPE,
                                )
                            elif bias_row is not None:
                                nc.vector.tensor_tensor(
                                    out=ev[:mw, n * 512 : n * 512 + nw],
                                    in0=pm[:mw, :nw],
                                    in1=bias_row[:mw, n * 512 : n * 512 + nw],
                                    op=mybir.AluOpType.add,
                                )
                            else:
                                nc.vector.tensor_copy(
                                    ev[:mw, n * 512 : n * 512 + nw], pm[:mw, :nw]
                                )
                        nc.sync.dma_start(
                            g_dst[d0 + m4 * P : d0 + m4 * P + mw, :fo], ev[:mw, :fo]
                        )

            # dense (transposed out, L1): h1T[m, :] = Lrelu(W1[:,m]^T aggT1 + b1)
            def dense_t(src_sbs, kks, fo, w_tiles, dst_dram, bias_col):
                nk = len(src_sbs)
                nm = fo // P
                for d0 in range(0, NPC, 512):
                    dw = min(512, NPC - d0)
                    for m in range(nm):
                        pm = pmm.tile([P, 512], F32, name=f"apm_{d0}_{m}", tag="pmm", space="PSUM")
                        for k in range(nk):
                            kk = kks[k]
                            nc.tensor.matmul(
                                pm[:, :dw],
                                w_tiles[k][0][:kk, m * P : (m + 1) * P],
                                src_sbs[k][:kk, d0 : d0 + dw],
                                start=(k == 0), stop=(k == nk - 1),
                            )
                        ev = sb.tile([P, 512], DT, name=f"aev_{d0}_{m}", tag="ev")
                        nc.scalar.activation(
                            ev[:, :dw], pm[:, :dw], LRELU,
                            bias=bias_col[:, m : m + 1], alpha=NEG_SLOPE,
                        )
                        nc.sync.dma_start(dst_dram[m * P : (m + 1) * P, d0 : d0 + dw], ev[:, :dw])

            def allgather(src_d, dst_t):
                nc.gpsimd.collective_compute(
                    "AllGather", mybir.AluOpType.bypass, replica_groups=rg,
                    ins=[src_d[:].opt()], outs=[dst_t[:].opt()],
                )

            # ================= the network =================
            # L1: aggregate x (transposed, Copy evict) -> dense W1 -> h1T
            w1 = load_w(0)
            b1c = load_bcol(0)

            def l1_dense_block(d0, dw):
                for m in range(8):
                    pm = pmm.tile([P, 512], F32, name=f"apm_{d0}_{m}", tag="pmm", space="PSUM")
                    for k in range(3):
                        kk = (128, 128, 44)[k]
                        nc.tensor.matmul(
                            pm[:, :dw],
                            w1[k][0][:kk, m * P : (m + 1) * P],
                            aggT1_sb[k][:kk, d0 : d0 + dw],
                            start=(k == 0), stop=(k == 2),
                        )
                    ev = sb.tile([P, 512], DT, name=f"aev_{d0}_{m}", tag="ev")
                    nc.scalar.activation(
                        ev[:, :dw], pm[:, :dw], LRELU,
                        bias=b1c[:, m : m + 1], alpha=NEG_SLOPE,
                    )
                    nc.sync.dma_start(h1T_d[m * P : (m + 1) * P, d0 : d0 + dw], ev[:, :dw])

            def l1_out(t, tw, pts):
                for fc in range(3):
                    fw = min(P, 300 - fc * P)
                    nc.vector.tensor_copy(
                        aggT1_sb[fc][:fw, t * P : t * P + tw], pts[fc][:fw, :tw]
                    )
                if t % 4 == 3 or t == N_TILES - 1:
                    d0 = (t // 4) * 512
                    l1_dense_block(d0, t * P + tw - d0)

            aggregate(0, None, l1_out)

            # L2: dense W2 -> g2 -> AG -> aggregate (Lrelu+b2) -> h2T
            w2 = load_w(1)
            b2c = load_bcol(1)
            dense_n(1, h1T_d, 1024, 512, w2, g2_d, None, False)
            allgather(g2_d, T2)
            aggregate(1, T2[:, :], make_staged_out(1, 4, h2T_d, b2c, True))

            # L3
            w3 = load_w(2)
            b3c = load_bcol(2)
            dense_n(2, h2T_d, 512, 256, w3, g3_d, None, False)
            allgather(g3_d, T3)
            aggregate(2, T3[:, :], make_staged_out(2, 2, h3T_d, b3c, True))

            # L4: dense W4 -> g4 -> AG -> aggregate node-major (+b4, Lrelu) -> h4
            w4 = load_w(3)
            dense_n(3, h3T_d, 256, 128, w4, g4_d, None, False)
            allgather(g4_d, T4)

            def l4_out(t, tw, pts):
                ev = sb.tile([P, 512], DT, name=f"l4ev_{t}", tag="ev")
                nc.scalar.activation(ev[:tw, :128], pts[0][:tw, :128], LRELU, alpha=NEG_SLOPE)
                nc.scalar.dma_start(h4_d[t * P : t * P + tw, :], ev[:tw, :128])

            aggregate(3, T4[:, :], l4_out, node_major=True)

            # L5: AG h4 -> aggregate (Copy) -> aggT5 (SBUF) -> dense W5 (+b5) -> out
            allgather(h4_d, T5)

            w5 = load_w(4)

            def l5_dense_tile(t, tw):
                ev = sb.tile([P, 2048], F32, name=f"oev_{t}", tag="oev")
                for n in range(4):
                    pm = pmm.tile([P, 512], F32, name=f"pm5_{t}_{n}", tag="pmm", space="PSUM")
                    nc.tensor.matmul(
                        pm[:tw, :], aggT5_sb[:, t * P : t * P + tw],
                        w5[0][0][:, n * 512 : (n + 1) * 512],
                        start=True, stop=True,
                    )
                    nc.vector.tensor_tensor(
                        out=ev[:tw, n * 512 : (n + 1) * 512], in0=pm[:tw, :],
                        in1=b5r_sb[:tw, n * 512 : (n + 1) * 512],
                        op=mybir.AluOpType.add,
                    )
                nc.sync.dma_start(out_d[t * P : t * P + tw, :], ev[:tw, :])

            def l5_out(t, tw, pts):
                nc.vector.tensor_copy(aggT5_sb[:, t * P : t * P + tw], pts[0][:, :tw])
                l5_dense_tile(t, tw)

            aggregate(4, T5[:, :], l5_out)

            if debug:
                dbg = {
                    "h1T": h1T_d, "g2": g2_d, "h2T": h2T_d,
                    "g3": g3_d, "h3T": h3T_d, "g4": g4_d, "h4": h4_d,
                }
                for nm, t_ in dbg.items():
                    shp = list(t_.shape)
                    d_ = nc.declare_dram_parameter(f"dbg_{nm}", shp, F32, isOutput=True)
                    nc.sync.dma_start(d_[:], t_[:])

    nc.compile()
    return nc


# ----------------------------------------------------------------------------
# Entry point
# ----------------------------------------------------------------------------

_CACHE = {}


def _run(inputs, trace=False):
    x = np.asarray(inputs["x"], dtype=np.float32)
    edge_index = np.asarray(inputs["edge_index"])
    sched_lo, sched_hi, per_core = _prep(edge_index, x=x)

    key = (tuple(sched_lo.tolist()), tuple(sched_hi.tolist()))
    if key not in _CACHE:
        _CACHE[key] = _build(sched_lo, sched_hi)
    nc = _CACHE[key]

    common = {}
    for i in range(5):
        common[f"W{i+1}"] = np.ascontiguousarray(
            np.asarray(inputs[f"W{i+1}"], dtype=np.float32).astype(NPDT))
        common[f"b{i+1}"] = np.ascontiguousarray(
            np.asarray(inputs[f"b{i+1}"], dtype=np.float32).reshape(-1, 1))
    common["b4r"] = np.ascontiguousarray(common["b4"].reshape(1, 128).astype(NPDT))
    common["b5r"] = np.ascontiguousarray(np.broadcast_to(common["b5"].reshape(1, 2048), (128, 2048)).astype(np.float32))

    in_maps = [
        {**common, **{k: (v.astype(NPDT) if k.startswith("s_") else v)
                      for k, v in per_core[c].items()}}
        for c in range(NC)
    ]
    res = run_bass_kernel_spmd(nc, in_maps, core_ids=list(range(NC)), trace=trace)
    out = np.concatenate([res.results[c]["out"] for c in range(NC)], axis=0)
    return out, res


def kernel(**inputs):
    out, _ = _run(inputs, trace=False)
    return out


# revision 17
# speedup vs baseline: 1.0696x; 1.0696x over previous
"""5-layer DGL-style GraphConv (AwA2Conv) on 8 Trainium2 NeuronCores.

Math per layer (norm='both'):
    out = D_in^{-1/2} A D_out^{-1/2} (h) @ W + b     (+ leaky_relu except last)

The per-edge weight w_e = dinv_out[src]*dinv_in[dst] is folded into
block-sparse "S" matrices (128 edges x 128 dst one-hot-weighted) so the
sparse aggregation becomes PE matmuls over dma_gather'ed edge rows.
Aggregation runs at min(Fin, Fout) per layer (matmul commutes with the
linear aggregation). With lhsT = gathered rows, the aggregate comes out
TRANSPOSED [F, dst], which is exactly the lhsT layout the next dense matmul
needs — the whole network runs with zero explicit transposes.

Distribution: nodes sharded 6250/core; x and weights replicated via in_maps;
h exchanged between layers with AllGather; each core aggregates + transforms
its own 6250 destination nodes.
"""

import numpy as np

import concourse.bass as bass
import concourse.bacc as bacc
import concourse.mybir as mybir
import concourse.tile as tile
from concourse.bass_utils import run_bass_kernel_spmd

N_NODES = 50000
N_EDGES = 250000
NC = 8
NPC = N_NODES // NC  # 6250
P = 128
N_TILES = (NPC + P - 1) // P  # 49, last tile has 106 dsts
DIMS = [300, 1024, 512, 256, 128, 2048]
NEG_SLOPE = 0.01
LO_LIM = 32768  # int16 gather index limit

import ml_dtypes

F32 = mybir.dt.float32
BF16 = mybir.dt.bfloat16
DT = BF16                     # compute/storage dtype for activations/weights/S
NPDT = ml_dtypes.bfloat16
I16 = mybir.dt.int16
LRELU = mybir.ActivationFunctionType.Lrelu

LAYER_FA = [300, 512, 256, 128, 128]       # aggregation width
LAYER_FA_PAD = [384, 512, 256, 128, 128]   # table row width (256B multiple)
GROUP_T = 2                                # dst-tiles per dma_gather call


def _ceil_div(a, b):
    return (a + b - 1) // b


# ----------------------------------------------------------------------------
# Host-side graph preprocessing
# ----------------------------------------------------------------------------

def _prep(edge_index, x=None):
    """Partition edges by dst core/tile, split lo/hi src, pad to 16-granular
    per-tile schedules (max across cores -> one SPMD program).

    Returns (sched_lo, sched_hi, per_core): sched_* are per-tile padded idx
    counts (multiples of 16); per_core holds wrapped int16 gather indices,
    S matrices (128-row chunks, zero-padded), and the pre-gathered x rows
    for layer 1 (host-side halo materialization of the replicated input).
    """
    GRAN = 16
    src = np.asarray(edge_index[0], dtype=np.int64)
    dst = np.asarray(edge_index[1], dtype=np.int64)
    out_deg = np.bincount(src, minlength=N_NODES).astype(np.float32)
    in_deg = np.bincount(dst, minlength=N_NODES).astype(np.float32)
    dinv_out = 1.0 / np.sqrt(np.maximum(out_deg, 1.0))
    dinv_in = 1.0 / np.sqrt(np.maximum(in_deg, 1.0))
    w = (dinv_out[src] * dinv_in[dst]).astype(np.float32)

    lo = src < LO_LIM
    core_all = dst // NPC
    tile_all = (dst % NPC) // P
    key = ((core_all * N_TILES + tile_all) * 2 + (~lo).astype(np.int64))
    order = np.lexsort((src, key))
    src_s, w_s, dst_s, key_s = src[order], w[order], dst[order], key[order]
    bounds = np.searchsorted(key_s, np.arange(NC * N_TILES * 2 + 1))

    n_lo = np.zeros((NC, N_TILES), dtype=np.int64)
    n_hi = np.zeros((NC, N_TILES), dtype=np.int64)
    for c in range(NC):
        for t in range(N_TILES):
            k = (c * N_TILES + t) * 2
            n_lo[c, t] = bounds[k + 1] - bounds[k]
            n_hi[c, t] = bounds[k + 2] - bounds[k + 1]

    sched_lo = np.maximum(
        np.ceil(n_lo.max(axis=0) / GRAN).astype(np.int64), 1) * GRAN
    sched_hi = np.ceil(n_hi.max(axis=0) / GRAN).astype(np.int64) * GRAN

    xb = None if x is None else np.asarray(x, dtype=np.float32)

    per_core = []
    for c in range(NC):
        idx_parts = {True: [], False: []}
        s_parts = {True: [], False: []}
        xg_parts = {True: [], False: []}
        for t in range(N_TILES):
            k = (c * N_TILES + t) * 2
            segs = (
                (True, sched_lo[t], bounds[k], bounds[k + 1]),
                (False, sched_hi[t], bounds[k + 1], bounds[k + 2]),
            )
            for islo, ni, a, b_ in segs:
                ni = int(ni)
                if ni == 0:
                    continue
                n_slots = _ceil_div(ni, P) * P  # chunk-padded (S/xg rows)
                ne = b_ - a
                assert ne <= ni
                idx = np.zeros(ni, dtype=np.int64)
                idx[:ne] = src_s[a:b_] - (0 if islo else LO_LIM)
                dstloc = np.full(n_slots, P, dtype=np.int64)
                dstloc[:ne] = dst_s[a:b_] % NPC - t * P
                wv = np.zeros(n_slots, dtype=np.float32)
                wv[:ne] = w_s[a:b_]
                S = np.zeros((n_slots, P), dtype=np.float32)
                valid = dstloc < P
                S[np.nonzero(valid)[0], dstloc[valid]] = wv[valid]
                s_parts[islo].append(S.reshape(-1, P, P))
                idx_parts[islo].append(idx.reshape(-1, 16).T.astype(np.int16))
                if xb is not None:
                    xg = np.zeros((n_slots, 384), dtype=NPDT)
                    xg[:ne, :300] = xb[src_s[a:b_]].astype(NPDT)
                    xg_parts[islo].append(xg.reshape(-1, P, 384))
        idx_lo = np.tile(np.concatenate(idx_parts[True], axis=1), (8, 1))
        s_lo = np.concatenate(s_parts[True], axis=0)
        if idx_parts[False]:
            idx_hi = np.tile(np.concatenate(idx_parts[False], axis=1), (8, 1))
            s_hi = np.concatenate(s_parts[False], axis=0)
        else:
            idx_hi = np.zeros((128, 1), dtype=np.int16)
            s_hi = np.zeros((1, P, P), dtype=np.float32)
        pc = dict(
            idx_lo=np.ascontiguousarray(idx_lo),
            idx_hi=np.ascontiguousarray(idx_hi),
            s_lo=np.ascontiguousarray(s_lo),
            s_hi=np.ascontiguousarray(s_hi),
        )
        if xb is not None:
            pc["xg_lo"] = np.ascontiguousarray(np.concatenate(xg_parts[True], axis=0))
            pc["xg_hi"] = (
                np.ascontiguousarray(np.concatenate(xg_parts[False], axis=0))
                if xg_parts[False]
                else np.zeros((1, P, 384), dtype=NPDT)
            )
        per_core.append(pc)
    return sched_lo, sched_hi, per_core


# ----------------------------------------------------------------------------
# Bass program builder (depends only on sched_lo / sched_hi)
# ----------------------------------------------------------------------------

def _build(sched_lo, sched_hi, debug=False):
    nc = bacc.Bacc("TRN2")
    ch_lo = np.ceil(sched_lo / P).astype(np.int64)   # chunks per tile
    ch_hi = np.ceil(sched_hi / P).astype(np.int64)
    idx_lo_cols = int(sched_lo.sum()) // 16
    idx_hi_cols = max(int(sched_hi.sum()) // 16, 1)
    tot_clo = int(ch_lo.sum())
    tot_chi = max(int(ch_hi.sum()), 1)
    offi_lo = np.concatenate([[0], np.cumsum(sched_lo // 16)]).astype(int)  # idx cols
    offi_hi = np.concatenate([[0], np.cumsum(sched_hi // 16)]).astype(int)
    offc_lo = np.concatenate([[0], np.cumsum(ch_lo)]).astype(int)           # chunks
    offc_hi = np.concatenate([[0], np.cumsum(ch_hi)]).astype(int)

    xg_lo_d = nc.declare_dram_parameter("xg_lo", [tot_clo, P, 384], DT, isOutput=False)
    xg_hi_d = nc.declare_dram_parameter("xg_hi", [tot_chi, P, 384], DT, isOutput=False)
    Ws, bs = [], []
    for i in range(5):
        fi, fo = DIMS[i], DIMS[i + 1]
        Ws.append(nc.declare_dram_parameter(f"W{i+1}", [fi, fo], DT, isOutput=False))
        bs.append(nc.declare_dram_parameter(f"b{i+1}", [fo, 1], F32, isOutput=False))
    b4r_d = nc.declare_dram_parameter("b4r", [1, 128], DT, isOutput=False)
    b5r_d = nc.declare_dram_parameter("b5r", [128, 2048], F32, isOutput=False)
    idx_lo_d = nc.declare_dram_parameter("idx_lo", [128, idx_lo_cols], I16, isOutput=False)
    idx_hi_d = nc.declare_dram_parameter("idx_hi", [128, idx_hi_cols], I16, isOutput=False)
    s_lo_d = nc.declare_dram_parameter("s_lo", [tot_clo, P, P], DT, isOutput=False)
    s_hi_d = nc.declare_dram_parameter("s_hi", [tot_chi, P, P], DT, isOutput=False)
    out_d = nc.declare_dram_parameter("out", [NPC, 2048], F32, isOutput=True)

    with tile.TileContext(nc) as tc:
        with (
            tc.tile_pool(name="dram", bufs=1, space="DRAM") as dram,
            tc.tile_pool(name="cpool", bufs=1) as cpool,
            tc.tile_pool(name="sb", bufs=2) as sb,
            tc.tile_pool(name="pagg", bufs=1, space="PSUM") as pagg,
            tc.tile_pool(name="pmm", bufs=2, space="PSUM") as pmm,
        ):
            # ---- internal DRAM ----
            h1T_d = dram.tile([1024, NPC], DT)
            h2T_d = dram.tile([512, NPC], DT)
            h3T_d = dram.tile([256, NPC], DT)
            g2_d = dram.tile([NPC, 512], DT)
            g3_d = dram.tile([NPC, 256], DT)
            g4_d = dram.tile([NPC, 128], DT)
            h4_d = dram.tile([NPC, 128], DT)
            T2 = dram.tile([N_NODES, 512], DT, addr_space="Shared")
            T3 = dram.tile([N_NODES, 256], DT, addr_space="Shared")
            T4 = dram.tile([N_NODES, 128], DT, addr_space="Shared")
            T5 = dram.tile([N_NODES, 128], DT, addr_space="Shared")

            # ---- resident SBUF ----
            idx_lo_sb = cpool.tile([128, idx_lo_cols], I16, name="idxlo")
            nc.sync.dma_start(idx_lo_sb[:], idx_lo_d[:])
            idx_hi_sb = cpool.tile([128, idx_hi_cols], I16, name="idxhi")
            nc.sync.dma_start(idx_hi_sb[:], idx_hi_d[:])
            aggT1_sb = [
                cpool.tile([P, NPC], DT, name=f"aggT1_{k}") for k in range(3)
            ]
            aggT5_sb = cpool.tile([P, NPC], DT, name="aggT5")
            ones_sb = cpool.tile([1, 128], DT, name="ones")
            nc.any.memset(ones_sb[:], 1.0)
            b4r_sb = cpool.tile([1, 128], DT, name="b4rsb")
            nc.sync.dma_start(b4r_sb[:], b4r_d[:])
            b5r_sb = cpool.tile([128, 2048], F32, name="b5rsb")
            nc.sync.dma_start(b5r_sb[:], b5r_d[:])

            rg = [list(range(NC))]

            def load_w(i):
                fi, fo = DIMS[i], DIMS[i + 1]
                ks = []
                for k in range(_ceil_div(fi, P)):
                    kk = min(P, fi - k * P)
                    t_ = cpool.tile([P, fo], DT, name=f"w{i}_{k}", tag=f"wk{k}")
                    nc.sync.dma_start(t_[:kk, :], Ws[i][k * P : k * P + kk, :])
                    ks.append((t_, kk))
                return ks

            def load_bcol(i):
                fo = DIMS[i + 1]
                nchunk = _ceil_div(fo, P)
                t_ = cpool.tile([P, 16], F32, name=f"bc{i}", tag="bcol")
                for m in range(nchunk):
                    mm = min(P, fo - m * P)
                    nc.sync.dma_start(t_[:mm, m : m + 1], bs[i][m * P : m * P + mm, :])
                return t_

            # ================= aggregation =================
            def aggregate(layer, table_ap, out_cb, node_major=False):
                """Gather + aggregate all dst tiles.

                layer 0 reads host-shipped pre-gathered x rows (xg_*) with a
                plain DMA; other layers dma_gather from table_ap. S matrices
                are zero-padded to 128-row chunks; gathers are 16-granular
                with partial-K matmuls on the last chunk of each tile.
                """
                fa = LAYER_FA[layer]
                fap = LAYER_FA_PAD[layer]
                nfc = _ceil_div(fa, P)
                for g0 in range(0, N_TILES, 4):
                    tiles = list(range(g0, min(g0 + 4, N_TILES)))
                    t0, t1 = tiles[0], tiles[-1]
                    # S loads, one per group (contiguous chunk ranges)
                    slo_sb = sb.tile([128, int(offc_lo[t1 + 1] - offc_lo[t0]), P], DT,
                                     name=f"slo_{layer}_{g0}", tag="slo")
                    nc.sync.dma_start(
                        slo_sb[:],
                        s_lo_d[offc_lo[t0] : offc_lo[t1 + 1]].rearrange("c p n -> p c n"),
                    )
                    g_chi = int(offc_hi[t1 + 1] - offc_hi[t0])
                    shi_sb = None
                    if g_chi > 0:
                        shi_sb = sb.tile([128, g_chi, P], DT, name=f"shi_{layer}_{g0}", tag="shi")
                        nc.sync.dma_start(
                            shi_sb[:],
                            s_hi_d[offc_hi[t0] : offc_hi[t1 + 1]].rearrange("c p n -> p c n"),
                        )
                    if layer == 0:
                        # pre-gathered rows: one DMA per group per half
                        clo_g = int(offc_lo[t1 + 1] - offc_lo[t0])
                        hg_lo_g = sb.tile([128, clo_g, fap], DT, name=f"hglo_{layer}_{g0}", tag="hglo")
                        nc.sync.dma_start(
                            hg_lo_g[:],
                            xg_lo_d[offc_lo[t0] : offc_lo[t1 + 1]].rearrange("c p n -> p c n"),
                        )
                        hg_hi_g = None
                        if g_chi > 0:
                            hg_hi_g = sb.tile([128, g_chi, fap], DT, name=f"hghi_{layer}_{g0}", tag="hghi")
                            nc.sync.dma_start(
                                hg_hi_g[:],
                                xg_hi_d[offc_hi[t0] : offc_hi[t1 + 1]].rearrange("c p n -> p c n"),
                            )
                    for t in tiles:
                        tw = min(P, NPC - t * P)
                        # chunk list: (hg_tile, local_chunk, S_tile, s_chunk, K)
                        chunks = []
                        if layer == 0:
                            ni = int(sched_lo[t])
                            for ci in range(int(ch_lo[t])):
                                chunks.append((hg_lo_g, int(offc_lo[t] - offc_lo[t0]) + ci,
                                               slo_sb, int(offc_lo[t] - offc_lo[t0]) + ci, P))
                            ni = int(sched_hi[t])
                            for ci in range(int(ch_hi[t])):
                                chunks.append((hg_hi_g, int(offc_hi[t] - offc_hi[t0]) + ci,
                                               shi_sb, int(offc_hi[t] - offc_hi[t0]) + ci, P))
                        else:
                            ni_lo = int(sched_lo[t])
                            hg_lo = sb.tile([128, int(ch_lo[t]), fap], DT,
                                            name=f"hglo_{layer}_{t}", tag="hglo")
                            nc.gpsimd.dma_gather(
                                hg_lo[:], table_ap,
                                idx_lo_sb[:, offi_lo[t] : offi_lo[t + 1]],
                                ni_lo, ni_lo, fap,
                            )
                            for ci in range(int(ch_lo[t])):
                                chunks.append((hg_lo, ci, slo_sb,
                                               int(offc_lo[t] - offc_lo[t0]) + ci,
                                               min(P, ni_lo - ci * P)))
                            ni_hi = int(sched_hi[t])
                            if ni_hi > 0:
                                hg_hi = sb.tile([128, int(ch_hi[t]), fap], DT,
                                                name=f"hghi_{layer}_{t}", tag="hghi")
                                nc.gpsimd.dma_gather(
                                    hg_hi[:], table_ap[LO_LIM:, :],
                                    idx_hi_sb[:, offi_hi[t] : offi_hi[t + 1]],
                                    ni_hi, ni_hi, fap,
                                )
                                for ci in range(int(ch_hi[t])):
                                    chunks.append((hg_hi, ci, shi_sb,
                                                   int(offc_hi[t] - offc_hi[t0]) + ci,
                                                   min(P, ni_hi - ci * P)))
                        # one PSUM bank per accumulation group: first_mm's
                        # has_written clear is (partition-row x bank)-granular
                        pts = [
                            pagg.tile([P, P], F32, name=f"pt_{layer}_{t}_{fc}",
                                      tag=f"pagg{fc}", space="PSUM",
                                      bufs=(2 if fc < 2 else 1))
                            for fc in range(nfc)
                        ]
                        nch = len(chunks)
                        if node_major:
                            for ci, (hg, hc, ssb, sc, K) in enumerate(chunks):
                                nc.tensor.matmul(
                                    pts[0][:, :fa], ssb[:K, sc, :], hg[:K, hc, :fa],
                                    start=(ci == 0), stop=False,
                                )
                            nc.tensor.matmul(  # += bias row
                                pts[0][:, :fa], ones_sb[:1, :], b4r_sb[:1, :fa],
                                start=False, stop=True,
                            )
                        else:
                            for ci, (hg, hc, ssb, sc, K) in enumerate(chunks):
                                for fc in range(nfc):
                                    fw = min(P, fa - fc * P)
                                    nc.tensor.matmul(
                                        pts[fc][:fw, :],
                                        hg[:K, hc, fc * P : fc * P + fw],
                                        ssb[:K, sc, :],
                                        start=(ci == 0), stop=(ci == nch - 1),
                                    )
                        out_cb(t, tw, pts)

            # transposed-agg eviction via staging tiles flushed every 4 dst-tiles
            def make_staged_out(layer, nfc, dst_dram, bias_col, lrelu):
                state = {"stages": None, "c0": 0}

                def flush(c0, cw):
                    for fc in range(nfc):
                        nc.scalar.dma_start(
                            dst_dram[fc * P : (fc + 1) * P, c0 : c0 + cw],
                            state["stages"][fc][:, :cw],
                        )

                def cb(t, tw, pts):
                    if t % 4 == 0:
                        state["stages"] = [
                            sb.tile([P, 512], DT, name=f"st_{layer}_{t}_{fc}", tag=f"st{fc}")
                            for fc in range(nfc)
                        ]
                        state["c0"] = t * P
                    col = (t % 4) * P
                    for fc in range(nfc):
                        if lrelu:
                            nc.scalar.activation(
                                state["stages"][fc][:, col : col + tw],
                                pts[fc][:, :tw],
                                LRELU, bias=bias_col[:, fc : fc + 1], alpha=NEG_SLOPE,
                            )
                        else:
                            nc.vector.tensor_copy(
                                state["stages"][fc][:, col : col + tw],
                                pts[fc][:, :tw],
                            )
                    if t % 4 == 3 or t == N_TILES - 1:
                        flush(state["c0"], t * P + tw - state["c0"])

                return cb

            # dense (node-major out): g[n, :] = hT[:, n]^T @ W (+bias) (+lrelu)
            def dense_n(li, hT_src, fi, fo, w_tiles, g_dst, bias_row, lrelu):
                nk = fi // P
                nn = _ceil_div(fo, 512)
                src_is_sb = hT_src.space == bass.MemorySpace.SBUF
                assert not src_is_sb or nk == 1
                for d0 in range(0, NPC, 512):
                    dw = min(512, NPC - d0)
                    if src_is_sb:
                        hsb = None
                    else:
                        hsb = sb.tile([128, nk, 512], DT, name=f"hsb_{li}_{d0}", tag="hsb")
                        nc.sync.dma_start(
                            hsb[:, :, :dw],
                            hT_src[:, d0 : d0 + dw].rearrange("(k p) c -> p k c", p=P),
                        )
                    for m4 in range(_ceil_div(dw, P)):
                        mw = min(P, dw - m4 * P)
                        if fo > 512:
                            ev = sb.tile([P, 2048], F32, name=f"oev_{li}_{d0}_{m4}", tag="oev")
                        else:
                            ev = sb.tile([P, 512], DT, name=f"ev_{li}_{d0}_{m4}", tag="ev")
                        for n in range(nn):
                            nw = min(512, fo - n * 512)
                            pm = pmm.tile([P, 512], F32, name=f"pm_{li}_{d0}_{m4}_{n}",
                                          tag="pmm", space="PSUM")
                            for k in range(nk):
                                lhs = (
                                    hT_src[:, d0 + m4 * P : d0 + m4 * P + mw]
                                    if src_is_sb
                                    else hsb[:, k, m4 * P : m4 * P + mw]
                                )
                                nc.tensor.matmul(
                                    pm[:mw, :nw],
                                    lhs,
                                    w_tiles[k][0][:w_tiles[k][1], n * 512 : n * 512 + nw],
                                    start=(k == 0), stop=(k == nk - 1),
                                )
                            if lrelu:
                                nc.scalar.activation(
                                    ev[:mw, n * 512 : n * 512 + nw], pm[:mw, :nw],
                                    LRELU, alpha=NEG_SLOPE,
                                )
                            elif bias_row is not None:
                                nc.vector.tensor_tensor(
                                    out=ev[:mw, n * 512 : n * 512 + nw],
                                    in0=pm[:mw, :nw],
                                    in1=bias_row[:mw, n * 512 : n * 512 + nw],
                                    op=mybir.AluOpType.add,
                                )
                            else:
                                nc.vector.tensor_copy(
                                    ev[:mw, n * 512 : n * 512 + nw], pm[:mw, :nw]
                                )
                        nc.sync.dma_start(
                            g_dst[d0 + m4 * P : d0 + m4 * P + mw, :fo], ev[:mw, :fo]
                        )

            # dense (transposed out, L1): h1T[m, :] = Lrelu(W1[:,m]^T aggT1 + b1)
            def dense_t(src_sbs, kks, fo, w_tiles, dst_dram, bias_col):
                nk = len(src_sbs)
                nm = fo // P
                for d0 in range(0, NPC, 512):
                    dw = min(512, NPC - d0)
                    for m in range(nm):
                        pm = pmm.tile([P, 512], F32, name=f"apm_{d0}_{m}", tag="pmm", space="PSUM")
                        for k in range(nk):
                            kk = kks[k]
                            nc.tensor.matmul(
                                pm[:, :dw],
                                w_tiles[k][0][:kk, m * P : (m + 1) * P],
                                src_sbs[k][:kk, d0 : d0 + dw],
                                start=(k == 0), stop=(k == nk - 1),
                            )
                        ev = sb.tile([P, 512], DT, name=f"aev_{d0}_{m}", tag="ev")
                        nc.scalar.activation(
                            ev[:, :dw], pm[:, :dw], LRELU,
                            bias=bias_col[:, m : m + 1], alpha=NEG_SLOPE,
                        )
                        nc.sync.dma_start(dst_dram[m * P : (m + 1) * P, d0 : d0 + dw], ev[:, :dw])

            def allgather(src_d, dst_t):
                nc.gpsimd.collective_compute(
                    "AllGather", mybir.AluOpType.bypass, replica_groups=rg,
                    ins=[src_d[:].opt()], outs=[dst_t[:].opt()],
                )

            # ================= the network =================
            # L1: aggregate x (transposed, Copy evict) -> dense W1 -> h1T
            w1 = load_w(0)
            b1c = load_bcol(0)

            def l1_dense_block(d0, dw):
                for m in range(8):
                    pm = pmm.tile([P, 512], F32, name=f"apm_{d0}_{m}", tag="pmm", space="PSUM")
                    for k in range(3):
                        kk = (128, 128, 44)[k]
                        nc.tensor.matmul(
                            pm[:, :dw],
                            w1[k][0][:kk, m * P : (m + 1) * P],
                            aggT1_sb[k][:kk, d0 : d0 + dw],
                            start=(k == 0), stop=(k == 2),
                        )
                    ev = sb.tile([P, 512], DT, name=f"aev_{d0}_{m}", tag="ev")
                    nc.scalar.activation(
                        ev[:, :dw], pm[:, :dw], LRELU,
                        bias=b1c[:, m : m + 1], alpha=NEG_SLOPE,
                    )
                    nc.sync.dma_start(h1T_d[m * P : (m + 1) * P, d0 : d0 + dw], ev[:, :dw])

            def l1_out(t, tw, pts):
                for fc in range(3):
                    fw = min(P, 300 - fc * P)
                    nc.vector.tensor_copy(
                        aggT1_sb[fc][:fw, t * P : t * P + tw], pts[fc][:fw, :tw]
                    )
                if t % 4 == 3 or t == N_TILES - 1:
                    d0 = (t // 4) * 512
                    l1_dense_block(d0, t * P + tw - d0)

            aggregate(0, None, l1_out)

            # L2: dense W2 -> g2 -> AG -> aggregate (Lrelu+b2) -> h2T
            w2 = load_w(1)
            b2c = load_bcol(1)
            dense_n(1, h1T_d, 1024, 512, w2, g2_d, None, False)
            allgather(g2_d, T2)
            aggregate(1, T2[:, :], make_staged_out(1, 4, h2T_d, b2c, True))

            # L3
            w3 = load_w(2)
            b3c = load_bcol(2)
            dense_n(2, h2T_d, 512, 256, w3, g3_d, None, False)
            allgather(g3_d, T3)
            aggregate(2, T3[:, :], make_staged_out(2, 2, h3T_d, b3c, True))

            # L4: dense W4 -> g4 -> AG -> aggregate node-major (+b4, Lrelu) -> h4
            w4 = load_w(3)
            dense_n(3, h3T_d, 256, 128, w4, g4_d, None, False)
            allgather(g4_d, T4)

            def l4_out(t, tw, pts):
                ev = sb.tile([P, 512], DT, name=f"l4ev_{t}", tag="ev")
                nc.scalar.activation(ev[:tw, :128], pts[0][:tw, :128], LRELU, alpha=NEG_SLOPE)
                nc.scalar.dma_start(h4_d[t * P : t * P + tw, :], ev[:tw, :128])

            aggregate(3, T4[:, :], l4_out, node_major=True)

            # L5: AG h4 -> aggregate (Copy) -> aggT5 (SBUF) -> dense W5 (+b5) -> out
            allgather(h4_d, T5)

            w5 = load_w(4)

            def l5_dense_tile(t, tw):
                ev = sb.tile([P, 2048], F32, name=f"oev_{t}", tag="oev")
                for n in range(4):
                    pm = pmm.tile([P, 512], F32, name=f"pm5_{t}_{n}", tag="pmm", space="PSUM")
                    nc.tensor.matmul(
                        pm[:tw, :], aggT5_sb[:, t * P : t * P + tw],
                        w5[0][0][:, n * 512 : (n + 1) * 512],
                        start=True, stop=True,
                    )
                    nc.vector.tensor_tensor(
                        out=ev[:tw, n * 512 : (n + 1) * 512], in0=pm[:tw, :],
                        in1=b5r_sb[:tw, n * 512 : (n + 1) * 512],
                        op=mybir.AluOpType.add,
                    )
                nc.sync.dma_start(out_d[t * P : t * P + tw, :], ev[:tw, :])

            def l5_out(t, tw, pts):
                nc.vector.tensor_copy(aggT5_sb[:, t * P : t * P + tw], pts[0][:, :tw])

            aggregate(4, T5[:, :], l5_out)
            for t in range(N_TILES):
                l5_dense_tile(t, min(P, NPC - t * P))

            if debug:
                dbg = {
                    "h1T": h1T_d, "g2": g2_d, "h2T": h2T_d,
                    "g3": g3_d, "h3T": h3T_d, "g4": g4_d, "h4": h4_d,
                }
                for nm, t_ in dbg.items():
                    shp = list(t_.shape)
                    d_ = nc.declare_dram_parameter(f"dbg_{nm}", shp, F32, isOutput=True)
                    nc.sync.dma_start(d_[:], t_[:])

    nc.compile()
    return nc


# ----------------------------------------------------------------------------
# Entry point
# ----------------------------------------------------------------------------

_CACHE = {}


def _run(inputs, trace=False):
    x = np.asarray(inputs["x"], dtype=np.float32)
    edge_index = np.asarray(inputs["edge_index"])
    sched_lo, sched_hi, per_core = _prep(edge_index, x=x)

    key = (tuple(sched_lo.tolist()), tuple(sched_hi.tolist()))
    if key not in _CACHE:
        _CACHE[key] = _build(sched_lo, sched_hi)
    nc = _CACHE[key]

    common = {}
    for i in range(5):
        common[f"W{i+1}"] = np.ascontiguousarray(
            np.asarray(inputs[f"W{i+1}"], dtype=np.float32).astype(NPDT))
        common[f"b{i+1}"] = np.ascontiguousarray(
            np.asarray(inputs[f"b{i+1}"], dtype=np.float32).reshape(-1, 1))
    common["b4r"] = np.ascontiguousarray(common["b4"].reshape(1, 128).astype(NPDT))
    common["b5r"] = np.ascontiguousarray(np.broadcast_to(common["b5"].reshape(1, 2048), (128, 2048)).astype(np.float32))

    in_maps = [
        {**common, **{k: (v.astype(NPDT) if k.startswith("s_") else v)
                      for k, v in per_core[c].items()}}
        for c in range(NC)
    ]
    res = run_bass_kernel_spmd(nc, in_maps, core_ids=list(range(NC)), trace=trace)
    out = np.concatenate([res.results[c]["out"] for c in range(NC)], axis=0)
    return out, res


def kernel(**inputs):
    out, _ = _run(inputs, trace=False)
    return out


# revision 18
# speedup vs baseline: 1.0787x; 1.0084x over previous
"""5-layer DGL-style GraphConv (AwA2Conv) on 8 Trainium2 NeuronCores.

Math per layer (norm='both'):
    out = D_in^{-1/2} A D_out^{-1/2} (h) @ W + b     (+ leaky_relu except last)

The per-edge weight w_e = dinv_out[src]*dinv_in[dst] is folded into
block-sparse "S" matrices (128-edge x 128-dst chunks) so the sparse
aggregation becomes PE matmuls over dma_gather'ed edge rows. Aggregation
runs at min(Fin, Fout) per layer (matmul commutes with the linear
aggregation). With lhsT = gathered rows the aggregate comes out TRANSPOSED
[F, dst] — exactly the lhsT layout the next dense matmul wants, so the
network runs with zero explicit transposes.

Distribution: dual-block node sharding — core c owns global nodes
[c*3125,(c+1)*3125) u [25000+c*3125, 25000+(c+1)*3125). Each activation
exchange is TWO AllGathers (node halves A/B); gathers for edges with
src<25000 read table A and only wait on the first collective, overlapping
the second. Layer-1 edge rows (gathered from the replicated input x) are
materialized host-side and shipped per core.
"""

import numpy as np
import ml_dtypes

import concourse.bass as bass
import concourse.bacc as bacc
import concourse.mybir as mybir
import concourse.tile as tile
from concourse.bass_utils import run_bass_kernel_spmd

N_NODES = 50000
N_EDGES = 250000
NC = 8
NPC = N_NODES // NC      # 6250 nodes per core
HALF = 25000             # global half boundary (= lo/hi gather split)
HPC = HALF // NC         # 3125 nodes per core per half
P = 128
TPH = 25                 # tiles per half (24x128 + 1x53)
N_TILES = 2 * TPH        # 50 dst tiles per core
DIMS = [300, 1024, 512, 256, 128, 2048]
NEG_SLOPE = 0.01

F32 = mybir.dt.float32
BF16 = mybir.dt.bfloat16
DT = BF16
NPDT = ml_dtypes.bfloat16
I16 = mybir.dt.int16
LRELU = mybir.ActivationFunctionType.Lrelu

LAYER_FA = [300, 512, 256, 128, 128]       # aggregation width
LAYER_FA_PAD = [384, 512, 256, 128, 128]   # gathered row width (256B mult)


def _ceil_div(a, b):
    return (a + b - 1) // b


def _tile_start(t):
    return (t // TPH) * HPC + (t % TPH) * P


def _tile_width(t):
    return HPC - (TPH - 1) * P if (t % TPH) == TPH - 1 else P


TILE_STARTS = [_tile_start(t) for t in range(N_TILES)]
TILE_WIDTHS = [_tile_width(t) for t in range(N_TILES)]


# ----------------------------------------------------------------------------
# Host-side graph preprocessing
# ----------------------------------------------------------------------------

def _prep(edge_index, x):
    """Partition edges by (dst core, dst tile), split by src half, pad to
    16-granular per-tile schedules (max across cores -> one SPMD program).

    Returns (sched_lo, sched_hi, per_core). per_core: wrapped int16 gather
    indices, S chunk matrices, and pre-gathered layer-1 x rows.
    """
    GRAN = 16
    src = np.asarray(edge_index[0], dtype=np.int64)
    dst = np.asarray(edge_index[1], dtype=np.int64)
    out_deg = np.bincount(src, minlength=N_NODES).astype(np.float32)
    in_deg = np.bincount(dst, minlength=N_NODES).astype(np.float32)
    dinv_out = 1.0 / np.sqrt(np.maximum(out_deg, 1.0))
    dinv_in = 1.0 / np.sqrt(np.maximum(in_deg, 1.0))
    w = (dinv_out[src] * dinv_in[dst]).astype(np.float32)
    xb = np.asarray(x, dtype=np.float32)

    # dst -> (core, local pos); dual-block sharding
    d_half = dst // HALF
    d_rem = dst % HALF
    d_core = d_rem // HPC
    d_with = d_rem % HPC
    d_pos = d_with + d_half * HPC               # local position in [0, NPC)
    d_tile = d_half * TPH + np.minimum(d_with // P, TPH - 1)
    lo = src < HALF

    key = (d_core * N_TILES + d_tile) * 2 + (~lo).astype(np.int64)
    order = np.lexsort((src, key))
    src_s, w_s, pos_s, key_s = src[order], w[order], d_pos[order], key[order]
    bounds = np.searchsorted(key_s, np.arange(NC * N_TILES * 2 + 1))

    n_lo = np.zeros((NC, N_TILES), dtype=np.int64)
    n_hi = np.zeros((NC, N_TILES), dtype=np.int64)
    for c in range(NC):
        for t in range(N_TILES):
            k = (c * N_TILES + t) * 2
            n_lo[c, t] = bounds[k + 1] - bounds[k]
            n_hi[c, t] = bounds[k + 2] - bounds[k + 1]

    sched_lo = np.maximum(
        np.ceil(n_lo.max(axis=0) / GRAN).astype(np.int64), 1) * GRAN
    sched_hi = np.ceil(n_hi.max(axis=0) / GRAN).astype(np.int64) * GRAN

    per_core = []
    for c in range(NC):
        idx_parts = {True: [], False: []}
        s_parts = {True: [], False: []}
        xg_parts = {True: [], False: []}
        for t in range(N_TILES):
            k = (c * N_TILES + t) * 2
            segs = (
                (True, sched_lo[t], bounds[k], bounds[k + 1]),
                (False, sched_hi[t], bounds[k + 1], bounds[k + 2]),
            )
            for islo, ni, a, b_ in segs:
                ni = int(ni)
                if ni == 0:
                    continue
                n_slots = _ceil_div(ni, P) * P
                ne = b_ - a
                assert ne <= ni
                idx = np.zeros(ni, dtype=np.int64)
                idx[:ne] = src_s[a:b_] - (0 if islo else HALF)
                dstloc = np.full(n_slots, P, dtype=np.int64)
                dstloc[:ne] = pos_s[a:b_] - TILE_STARTS[t]
                wv = np.zeros(n_slots, dtype=np.float32)
                wv[:ne] = w_s[a:b_]
                S = np.zeros((n_slots, P), dtype=np.float32)
                valid = dstloc < P
                S[np.nonzero(valid)[0], dstloc[valid]] = wv[valid]
                s_parts[islo].append(S.reshape(-1, P, P))
                idx_parts[islo].append(idx.reshape(-1, 16).T.astype(np.int16))
                xg = np.zeros((n_slots, 384), dtype=NPDT)
                xg[:ne, :300] = xb[src_s[a:b_]].astype(NPDT)
                xg_parts[islo].append(xg.reshape(-1, P, 384))
        pc = {}
        for islo, nm in ((True, "lo"), (False, "hi")):
            if idx_parts[islo]:
                pc[f"idx_{nm}"] = np.ascontiguousarray(
                    np.tile(np.concatenate(idx_parts[islo], axis=1), (8, 1)))
                pc[f"s_{nm}"] = np.ascontiguousarray(
                    np.concatenate(s_parts[islo], axis=0))
                pc[f"xg_{nm}"] = np.ascontiguousarray(
                    np.concatenate(xg_parts[islo], axis=0))
            else:
                pc[f"idx_{nm}"] = np.zeros((128, 1), dtype=np.int16)
                pc[f"s_{nm}"] = np.zeros((1, P, P), dtype=np.float32)
                pc[f"xg_{nm}"] = np.zeros((1, P, 384), dtype=NPDT)
        per_core.append(pc)
    return sched_lo, sched_hi, per_core


# ----------------------------------------------------------------------------
# Bass program builder (depends only on sched_lo / sched_hi)
# ----------------------------------------------------------------------------

def _build(sched_lo, sched_hi):
    nc = bacc.Bacc("TRN2")
    ch_lo = np.ceil(sched_lo / P).astype(np.int64)
    ch_hi = np.ceil(sched_hi / P).astype(np.int64)
    idx_lo_cols = int(sched_lo.sum()) // 16
    idx_hi_cols = max(int(sched_hi.sum()) // 16, 1)
    tot_clo = int(ch_lo.sum())
    tot_chi = max(int(ch_hi.sum()), 1)
    offi_lo = np.concatenate([[0], np.cumsum(sched_lo // 16)]).astype(int)
    offi_hi = np.concatenate([[0], np.cumsum(sched_hi // 16)]).astype(int)
    offc_lo = np.concatenate([[0], np.cumsum(ch_lo)]).astype(int)
    offc_hi = np.concatenate([[0], np.cumsum(ch_hi)]).astype(int)

    xg_lo_d = nc.declare_dram_parameter("xg_lo", [tot_clo, P, 384], DT, isOutput=False)
    xg_hi_d = nc.declare_dram_parameter("xg_hi", [tot_chi, P, 384], DT, isOutput=False)
    Ws, bs = [], []
    for i in range(5):
        fi, fo = DIMS[i], DIMS[i + 1]
        Ws.append(nc.declare_dram_parameter(f"W{i+1}", [fi, fo], DT, isOutput=False))
        bs.append(nc.declare_dram_parameter(f"b{i+1}", [fo, 1], F32, isOutput=False))
    b4r_d = nc.declare_dram_parameter("b4r", [1, 128], DT, isOutput=False)
    b5r_d = nc.declare_dram_parameter("b5r", [128, 2048], F32, isOutput=False)
    idx_lo_d = nc.declare_dram_parameter("idx_lo", [128, idx_lo_cols], I16, isOutput=False)
    idx_hi_d = nc.declare_dram_parameter("idx_hi", [128, idx_hi_cols], I16, isOutput=False)
    s_lo_d = nc.declare_dram_parameter("s_lo", [tot_clo, P, P], DT, isOutput=False)
    s_hi_d = nc.declare_dram_parameter("s_hi", [tot_chi, P, P], DT, isOutput=False)
    out_d = nc.declare_dram_parameter("out", [NPC, 2048], F32, isOutput=True)

    with tile.TileContext(nc) as tc:
        with (
            tc.tile_pool(name="dram", bufs=1, space="DRAM") as dram,
            tc.tile_pool(name="cpool", bufs=1) as cpool,
            tc.tile_pool(name="sb", bufs=2) as sb,
            tc.tile_pool(name="pagg", bufs=1, space="PSUM") as pagg,
            tc.tile_pool(name="pmm", bufs=2, space="PSUM") as pmm,
        ):
            # ---- internal DRAM ----
            h1T_d = dram.tile([1024, NPC], DT)
            h2T_d = dram.tile([512, NPC], DT)
            h3T_d = dram.tile([256, NPC], DT)
            g2_d = dram.tile([NPC, 512], DT)
            g3_d = dram.tile([NPC, 256], DT)
            g4_d = dram.tile([NPC, 128], DT)
            h4_d = dram.tile([NPC, 128], DT)
            # split tables: A = nodes [0, 25000), B = nodes [25000, 50000)
            T2a = dram.tile([HALF, 512], DT, addr_space="Shared")
            T2b = dram.tile([HALF, 512], DT, addr_space="Shared")
            T3a = dram.tile([HALF, 256], DT, addr_space="Shared")
            T3b = dram.tile([HALF, 256], DT, addr_space="Shared")
            T4a = dram.tile([HALF, 128], DT, addr_space="Shared")
            T4b = dram.tile([HALF, 128], DT, addr_space="Shared")
            T5a = dram.tile([HALF, 128], DT, addr_space="Shared")
            T5b = dram.tile([HALF, 128], DT, addr_space="Shared")

            # ---- resident SBUF ----
            aggT1_sb = [cpool.tile([P, NPC], DT, name=f"aggT1_{k}") for k in range(3)]
            aggT5_sb = cpool.tile([P, NPC], DT, name="aggT5")
            ones_sb = cpool.tile([1, 128], DT, name="ones")
            nc.any.memset(ones_sb[:], 1.0)
            b4r_sb = cpool.tile([1, 128], DT, name="b4rsb")
            nc.sync.dma_start(b4r_sb[:], b4r_d[:])
            b5r_sb = cpool.tile([128, 2048], F32, name="b5rsb")
            nc.sync.dma_start(b5r_sb[:], b5r_d[:])
            idx_lo_sb = cpool.tile([128, idx_lo_cols], I16, name="idxlo")
            nc.sync.dma_start(idx_lo_sb[:], idx_lo_d[:])
            idx_hi_sb = cpool.tile([128, idx_hi_cols], I16, name="idxhi")
            nc.sync.dma_start(idx_hi_sb[:], idx_hi_d[:])

            rg = [list(range(NC))]

            def load_w(i):
                fi, fo = DIMS[i], DIMS[i + 1]
                ks = []
                for k in range(_ceil_div(fi, P)):
                    kk = min(P, fi - k * P)
                    t_ = cpool.tile([P, fo], DT, name=f"w{i}_{k}", tag=f"wk{k}")
                    nc.sync.dma_start(t_[:kk, :], Ws[i][k * P : k * P + kk, :])
                    ks.append((t_, kk))
                return ks

            def load_bcol(i):
                fo = DIMS[i + 1]
                nchunk = _ceil_div(fo, P)
                t_ = cpool.tile([P, 16], F32, name=f"bc{i}", tag="bcol")
                for m in range(nchunk):
                    mm = min(P, fo - m * P)
                    nc.sync.dma_start(t_[:mm, m : m + 1], bs[i][m * P : m * P + mm, :])
                return t_

            def allgather2(src_d, dst_a, dst_b):
                nc.gpsimd.collective_compute(
                    "AllGather", mybir.AluOpType.bypass, replica_groups=rg,
                    ins=[src_d[:HPC, :].opt()], outs=[dst_a[:].opt()],
                )
                nc.gpsimd.collective_compute(
                    "AllGather", mybir.AluOpType.bypass, replica_groups=rg,
                    ins=[src_d[HPC:, :].opt()], outs=[dst_b[:].opt()],
                )

            # ================= aggregation =================
            def aggregate(layer, tab_a, tab_b, out_cb, node_major=False):
                """Gather + aggregate all dst tiles.

                layer 0 reads host-shipped pre-gathered x rows via plain DMA;
                other layers dma_gather rows from tab_a (src<25000) / tab_b.
                16-granular schedules, partial-K matmuls on last chunks.
                out_cb(t, tw, pts): per-F-chunk PSUM banks for dst tile t.
                """
                fa = LAYER_FA[layer]
                fap = LAYER_FA_PAD[layer]
                nfc = _ceil_div(fa, P)
                for g0 in range(0, N_TILES, 4):
                    tiles = list(range(g0, min(g0 + 4, N_TILES)))
                    t0, t1 = tiles[0], tiles[-1]
                    slo_sb = sb.tile([128, int(offc_lo[t1 + 1] - offc_lo[t0]), P], DT,
                                     name=f"slo_{layer}_{g0}", tag="slo")
                    nc.sync.dma_start(
                        slo_sb[:],
                        s_lo_d[offc_lo[t0] : offc_lo[t1 + 1]].rearrange("c p n -> p c n"),
                    )
                    g_chi = int(offc_hi[t1 + 1] - offc_hi[t0])
                    shi_sb = None
                    if g_chi > 0:
                        shi_sb = sb.tile([128, g_chi, P], DT, name=f"shi_{layer}_{g0}", tag="shi")
                        nc.sync.dma_start(
                            shi_sb[:],
                            s_hi_d[offc_hi[t0] : offc_hi[t1 + 1]].rearrange("c p n -> p c n"),
                        )
                    if layer == 0:
                        clo_g = int(offc_lo[t1 + 1] - offc_lo[t0])
                        hg_lo_g = sb.tile([128, clo_g, fap], DT, name=f"hglo_{layer}_{g0}", tag="hglo")
                        nc.sync.dma_start(
                            hg_lo_g[:],
                            xg_lo_d[offc_lo[t0] : offc_lo[t1 + 1]].rearrange("c p n -> p c n"),
                        )
                        hg_hi_g = None
                        if g_chi > 0:
                            hg_hi_g = sb.tile([128, g_chi, fap], DT, name=f"hghi_{layer}_{g0}", tag="hghi")
                            nc.sync.dma_start(
                                hg_hi_g[:],
                                xg_hi_d[offc_hi[t0] : offc_hi[t1 + 1]].rearrange("c p n -> p c n"),
                            )
                    for t in tiles:
                        tw = TILE_WIDTHS[t]
                        chunks = []
                        if layer == 0:
                            for ci in range(int(ch_lo[t])):
                                c = int(offc_lo[t] - offc_lo[t0]) + ci
                                chunks.append((hg_lo_g, c, slo_sb, c, P))
                            for ci in range(int(ch_hi[t])):
                                c = int(offc_hi[t] - offc_hi[t0]) + ci
                                chunks.append((hg_hi_g, c, shi_sb, c, P))
                        else:
                            ni_lo = int(sched_lo[t])
                            hg_lo = sb.tile([128, int(ch_lo[t]), fap], DT,
                                            name=f"hglo_{layer}_{t}", tag="hglo")
                            nc.gpsimd.dma_gather(
                                hg_lo[:], tab_a,
                                idx_lo_sb[:, offi_lo[t] : offi_lo[t + 1]],
                                ni_lo, ni_lo, fap,
                            )
                            for ci in range(int(ch_lo[t])):
                                chunks.append((hg_lo, ci, slo_sb,
                                               int(offc_lo[t] - offc_lo[t0]) + ci,
                                               min(P, ni_lo - ci * P)))
                            ni_hi = int(sched_hi[t])
                            if ni_hi > 0:
                                hg_hi = sb.tile([128, int(ch_hi[t]), fap], DT,
                                                name=f"hghi_{layer}_{t}", tag="hghi")
                                nc.gpsimd.dma_gather(
                                    hg_hi[:], tab_b,
                                    idx_hi_sb[:, offi_hi[t] : offi_hi[t + 1]],
                                    ni_hi, ni_hi, fap,
                                )
                                for ci in range(int(ch_hi[t])):
                                    chunks.append((hg_hi, ci, shi_sb,
                                                   int(offc_hi[t] - offc_hi[t0]) + ci,
                                                   min(P, ni_hi - ci * P)))
                        # one PSUM bank per accumulation group (first_mm's
                        # has_written clear is partition-row x bank granular)
                        pts = [
                            pagg.tile([P, P], F32, name=f"pt_{layer}_{t}_{fc}",
                                      tag=f"pagg{fc}", space="PSUM",
                                      bufs=(2 if fc < 2 else 1))
                            for fc in range(nfc)
                        ]
                        nch = len(chunks)
                        if node_major:
                            for ci, (hg, hc, ssb, sc, K) in enumerate(chunks):
                                nc.tensor.matmul(
                                    pts[0][:, :fa], ssb[:K, sc, :], hg[:K, hc, :fa],
                                    start=(ci == 0), stop=False,
                                )
                            nc.tensor.matmul(  # += bias row
                                pts[0][:, :fa], ones_sb[:1, :], b4r_sb[:1, :fa],
                                start=False, stop=True,
                            )
                        else:
                            for ci, (hg, hc, ssb, sc, K) in enumerate(chunks):
                                for fc in range(nfc):
                                    fw = min(P, fa - fc * P)
                                    nc.tensor.matmul(
                                        pts[fc][:fw, :],
                                        hg[:K, hc, fc * P : fc * P + fw],
                                        ssb[:K, sc, :],
                                        start=(ci == 0), stop=(ci == nch - 1),
                                    )
                        out_cb(t, tw, pts)

            # staged eviction (transposed agg -> hT dram), flushed per 4 tiles
            def make_staged_out(layer, nfc, dst_dram, bias_col, lrelu):
                state = {"stages": None, "c0": 0, "col": 0}

                def cb(t, tw, pts):
                    if t % 4 == 0:
                        state["stages"] = [
                            sb.tile([P, 512], DT, name=f"st_{layer}_{t}_{fc}", tag=f"st{fc}")
                            for fc in range(nfc)
                        ]
                        state["c0"] = TILE_STARTS[t]
                        state["col"] = 0
                    col = state["col"]
                    for fc in range(nfc):
                        if lrelu:
                            nc.scalar.activation(
                                state["stages"][fc][:, col : col + tw],
                                pts[fc][:, :tw],
                                LRELU, bias=bias_col[:, fc : fc + 1], alpha=NEG_SLOPE,
                            )
                        else:
                            nc.vector.tensor_copy(
                                state["stages"][fc][:, col : col + tw],
                                pts[fc][:, :tw],
                            )
                    state["col"] = col + tw
                    if t % 4 == 3 or t == N_TILES - 1:
                        for fc in range(nfc):
                            nc.scalar.dma_start(
                                dst_dram[fc * P : (fc + 1) * P,
                                         state["c0"] : state["c0"] + state["col"]],
                                state["stages"][fc][:, : state["col"]],
                            )

                return cb

            # dense (node-major out): g[n, :] = hT[:, n]^T @ W
            def dense_n(li, hT_src, fi, fo, w_tiles, g_dst):
                nk = fi // P
                for d0 in range(0, NPC, 512):
                    dw = min(512, NPC - d0)
                    hsb = sb.tile([128, nk, 512], DT, name=f"hsb_{li}_{d0}", tag="hsb")
                    nc.sync.dma_start(
                        hsb[:, :, :dw],
                        hT_src[:, d0 : d0 + dw].rearrange("(k p) c -> p k c", p=P),
                    )
                    for m4 in range(_ceil_div(dw, P)):
                        mw = min(P, dw - m4 * P)
                        ev = sb.tile([P, 512], DT, name=f"ev_{li}_{d0}_{m4}", tag="ev")
                        pm = pmm.tile([P, 512], F32, name=f"pm_{li}_{d0}_{m4}",
                                      tag="pmm", space="PSUM")
                        for k in range(nk):
                            nc.tensor.matmul(
                                pm[:mw, :fo],
                                hsb[:, k, m4 * P : m4 * P + mw],
                                w_tiles[k][0][:w_tiles[k][1], :fo],
                                start=(k == 0), stop=(k == nk - 1),
                            )
                        nc.vector.tensor_copy(ev[:mw, :fo], pm[:mw, :fo])
                        nc.sync.dma_start(
                            g_dst[d0 + m4 * P : d0 + m4 * P + mw, :fo], ev[:mw, :fo]
                        )

            # ================= the network =================
            # L1: aggregate shipped x rows -> aggT1 (SBUF) -> dense W1 -> h1T
            w1 = load_w(0)
            b1c = load_bcol(0)

            def l1_dense_block(d0, d1):
                dw = d1 - d0
                for m in range(8):
                    pm = pmm.tile([P, 512], F32, name=f"apm_{d0}_{m}", tag="pmm", space="PSUM")
                    for k in range(3):
                        kk = (128, 128, 44)[k]
                        nc.tensor.matmul(
                            pm[:, :dw],
                            w1[k][0][:kk, m * P : (m + 1) * P],
                            aggT1_sb[k][:kk, d0 : d0 + dw],
                            start=(k == 0), stop=(k == 2),
                        )
                    ev = sb.tile([P, 512], DT, name=f"aev_{d0}_{m}", tag="ev")
                    nc.scalar.activation(
                        ev[:, :dw], pm[:, :dw], LRELU,
                        bias=b1c[:, m : m + 1], alpha=NEG_SLOPE,
                    )
                    nc.sync.dma_start(h1T_d[m * P : (m + 1) * P, d0 : d0 + dw], ev[:, :dw])

            l1_state = {"done": 0}

            def l1_out(t, tw, pts):
                for fc in range(3):
                    fw = min(P, 300 - fc * P)
                    nc.vector.tensor_copy(
                        aggT1_sb[fc][:fw, TILE_STARTS[t] : TILE_STARTS[t] + tw],
                        pts[fc][:fw, :tw],
                    )
                covered = TILE_STARTS[t] + tw
                nblk = covered // 512 if t < N_TILES - 1 else _ceil_div(NPC, 512)
                while l1_state["done"] < nblk:
                    d0 = l1_state["done"] * 512
                    l1_dense_block(d0, min(d0 + 512, NPC))
                    l1_state["done"] += 1

            aggregate(0, None, None, l1_out)

            # L2: dense W2 -> g2 -> AGx2 -> aggregate (Lrelu+b2) -> h2T
            w2 = load_w(1)
            b2c = load_bcol(1)
            dense_n(2, h1T_d, 1024, 512, w2, g2_d)
            allgather2(g2_d, T2a, T2b)
            aggregate(1, T2a[:, :], T2b[:, :], make_staged_out(1, 4, h2T_d, b2c, True))

            # L3
            w3 = load_w(2)
            b3c = load_bcol(2)
            dense_n(3, h2T_d, 512, 256, w3, g3_d)
            allgather2(g3_d, T3a, T3b)
            aggregate(2, T3a[:, :], T3b[:, :], make_staged_out(2, 2, h3T_d, b3c, True))

            # L4: dense W4 -> g4 -> AGx2 -> aggregate node-major (+b4, Lrelu) -> h4
            w4 = load_w(3)
            dense_n(4, h3T_d, 256, 128, w4, g4_d)
            allgather2(g4_d, T4a, T4b)

            def l4_out(t, tw, pts):
                ev = sb.tile([P, 512], DT, name=f"l4ev_{t}", tag="ev")
                nc.scalar.activation(ev[:tw, :128], pts[0][:tw, :128], LRELU, alpha=NEG_SLOPE)
                nc.scalar.dma_start(
                    h4_d[TILE_STARTS[t] : TILE_STARTS[t] + tw, :], ev[:tw, :128])

            aggregate(3, T4a[:, :], T4b[:, :], l4_out, node_major=True)

            # L5: AGx2 h4 -> aggregate -> aggT5 (SBUF) -> dense W5 (+b5) -> out
            allgather2(h4_d, T5a, T5b)
            w5 = load_w(4)

            def l5_out(t, tw, pts):
                nc.vector.tensor_copy(
                    aggT5_sb[:, TILE_STARTS[t] : TILE_STARTS[t] + tw], pts[0][:, :tw])

            aggregate(4, T5a[:, :], T5b[:, :], l5_out)

            for d in range(_ceil_div(NPC, P)):
                r0 = d * P
                rw = min(P, NPC - r0)
                ev = sb.tile([P, 2048], F32, name=f"oev_{d}", tag="oev")
                for n in range(4):
                    pm = pmm.tile([P, 512], F32, name=f"pm5_{d}_{n}", tag="pmm", space="PSUM")
                    nc.tensor.matmul(
                        pm[:rw, :], aggT5_sb[:, r0 : r0 + rw],
                        w5[0][0][:, n * 512 : (n + 1) * 512],
                        start=True, stop=True,
                    )
                    nc.vector.tensor_tensor(
                        out=ev[:rw, n * 512 : (n + 1) * 512], in0=pm[:rw, :],
                        in1=b5r_sb[:rw, n * 512 : (n + 1) * 512],
                        op=mybir.AluOpType.add,
                    )
                nc.sync.dma_start(out_d[r0 : r0 + rw, :], ev[:rw, :])

    nc.compile()
    return nc


# ----------------------------------------------------------------------------
# Entry point
# ----------------------------------------------------------------------------

_CACHE = {}


def _run(inputs, trace=False):
    x = np.asarray(inputs["x"], dtype=np.float32)
    edge_index = np.asarray(inputs["edge_index"])
    sched_lo, sched_hi, per_core = _prep(edge_index, x)

    key = (tuple(sched_lo.tolist()), tuple(sched_hi.tolist()))
    if key not in _CACHE:
        _CACHE[key] = _build(sched_lo, sched_hi)
    nc = _CACHE[key]

    common = {}
    for i in range(5):
        common[f"W{i+1}"] = np.ascontiguousarray(
            np.asarray(inputs[f"W{i+1}"], dtype=np.float32).astype(NPDT))
        common[f"b{i+1}"] = np.ascontiguousarray(
            np.asarray(inputs[f"b{i+1}"], dtype=np.float32).reshape(-1, 1))
    common["b4r"] = np.ascontiguousarray(common["b4"].reshape(1, 128).astype(NPDT))
    common["b5r"] = np.ascontiguousarray(
        np.broadcast_to(
            np.asarray(inputs["b5"], dtype=np.float32).reshape(1, 2048), (128, 2048)
        ).astype(np.float32))

    in_maps = [
        {**common, **{k: (v.astype(NPDT) if k.startswith("s_") else v)
                      for k, v in per_core[c].items()}}
        for c in range(NC)
    ]
    res = run_bass_kernel_spmd(nc, in_maps, core_ids=list(range(NC)), trace=trace)
    # reassemble: core c rows [0:HPC] -> global [c*HPC:(c+1)*HPC],
    #             rows [HPC:NPC] -> global [HALF + c*HPC : HALF + (c+1)*HPC]
    out = np.empty((N_NODES, 2048), dtype=np.float32)
    for c in range(NC):
        oc = res.results[c]["out"]
        out[c * HPC : (c + 1) * HPC] = oc[:HPC]
        out[HALF + c * HPC : HALF + (c + 1) * HPC] = oc[HPC:]
    return out, res


def kernel(**inputs):
    out, _ = _run(inputs, trace=False)
    return out
